# revision 29
# baseline (speedup 1.0000x reference)
"""Trainium2 Bass kernel for nn_Attention_20925080666453 (delta-softmax).

Computation (faithful to the torch module quirk):
    e = (Q @ K) / sqrt(512)            # [B,H,S,S]
    a = softmax(e, axis=1)             # softmax over the HEAD axis
    o = a @ V                          # [B,H,S,d]
    out = o.reshape(B, S, H*d)

Head-axis softmax is invariant to subtracting head 7's scores:
    delta_h = e_h - e_7 (h=0..6), delta_7 = 0
    a_h = exp(delta_h) / (1 + sum_{j<7} exp(delta_j)),  a_7 = r = 1/(1 + sum)
Each delta is ONE contraction-128 matmul with stacked operands
lhsT = [K_h; -K_7], rhs = [Q_h^T; Q_7^T]; head 7 needs no score matmul,
no exp, and no normalize-mul (its AV moving operand is r itself).

Sharding: 8 cores = batch (2) x query-chunk (4); no collectives.

Per-core pipeline per (s-block, t-tile) iteration:
  - 7 delta-score MMs -> PSUM [t=128, s=512], exp on ScalarE (scale fused,
    fp16 out, 3x FD-1024 + 1x FD-512 instructions).
  - exp_sb slot 7 is pre-primed to 1.0 (once per pool buffer), so the
    z4/z2 tree sums exp deltas AND the +1 in full-width 2x DVE adds.
  - fused custom DVE op: rcp = recip1(z2[0] + z2[1])  (bit-trick seed +
    one tuned Newton step, ~0.2% max rel err).
  - normalize-mul over slots 0:7 with rcp broadcast; AV matmuls lag 2
    iterations and are interleaved pairwise between score matmuls.
"""

import os
import sys
import threading

sys.path.insert(0, "/opt/trn_rl_repo")

import numpy as np

import concourse.bacc as bacc
import concourse.bass as bass
import concourse.mybir as mybir
import concourse.tile as tile
from concourse.bass_utils import run_bass_kernel_spmd

# Problem dims
B, H, S, D = 2, 8, 4096, 64
HIDDEN = H * D
SCALE = float(1.0 / np.sqrt(np.float32(HIDDEN)))

P = 128              # partitions
NPAIR = H // 2       # head pairs
ND = H - 1           # delta heads (vs reference head 7)
N_CORES = 8
S_CHUNKS = 4         # query chunks per batch
S_LOC = S // S_CHUNKS    # 1024 queries per core
SBLK = 512               # s-block (one PSUM bank of fp32)
NSB = S_LOC // SBLK      # 2
NTT = S // P             # 32 key tiles of 128

EXP_BUFS = 6

_cache = {"nc": None}
_lock = threading.Lock()


def _register_recip_sum_op():
    """Register a fused custom DVE op: out = recip1(in0 + in1)."""
    import concourse.dve_ops as dve_ops
    from concourse.dve_ops import DveOp, OPS, _SUB_OPCODE_FOR_NAME, CUSTOM_DVE_SPECS
    from concourse.dve_spec import Spec, Src0, Src1, Bin, AluOp, C0, C1, lower
    from concourse.dve_uop import DveOpSpec
    from concourse.dve_table_gen import dve_ver_for

    name = "RECIP_SUM1_ANT"
    for op in OPS:
        if op.name == name:
            return op
    x = Src0 + Src1
    not_x = Bin(AluOp.BITWISE_NOT, x, x)
    y0 = not_x * C0
    body = y0 * (C1 - x * y0)

    def ref(in0, in1, c0, c1, c2):
        xx = np.asarray(in0, np.float32) + np.asarray(in1, np.float32)
        nx = (~xx.view(np.int32)).view(np.float32)
        yy0 = nx * c0
        return yy0 * (c1 - xx * yy0)

    spec = Spec(body=body, reference=ref)
    _SUB_OPCODE_FOR_NAME.setdefault(name, max(_SUB_OPCODE_FOR_NAME.values()) + 1)
    ver = dve_ver_for("TRN2")
    uops = lower(spec, ver=ver)
    sha = DveOpSpec(
        name=name, opcode=_SUB_OPCODE_FOR_NAME[name], uops=uops, rd1_en=True
    ).sha(ver)
    op = DveOp(name, spec, subdim=False, uops_sha={ver: sha})
    OPS.append(op)
    CUSTOM_DVE_SPECS[name] = spec
    return op


def _build():
    global _RSUM
    _RSUM = _register_recip_sum_op()
    nc = bacc.Bacc(
        "TRN2",
        target_bir_lowering=False,
        debug=False,
        enable_asserts=True,
        num_devices=N_CORES,
    )
    f32 = mybir.dt.float32
    f16 = mybir.dt.float16

    qh_d = nc.dram_tensor("QH", [ND, P, S_LOC], f16, kind="ExternalInput").ap()
    kh_d = nc.dram_tensor("KH", [ND, P, S], f16, kind="ExternalInput").ap()
    v_d = nc.dram_tensor("V", [H, P, NTT, D], f16, kind="ExternalInput").ap()
    out_d = nc.dram_tensor("OUT", [NPAIR, P, S_LOC], f32, kind="ExternalOutput").ap()

    Exp = mybir.ActivationFunctionType.Exp

    with tile.TileContext(nc) as tc:
        with (
            tc.tile_pool(name="consts", bufs=1) as consts,
            tc.tile_pool(name="score", bufs=2, space="PSUM") as score_pool,
            tc.tile_pool(name="oaccp", bufs=1, space="PSUM") as oacc_pool,
            tc.tile_pool(name="expp", bufs=EXP_BUFS) as exp_pool,
            tc.tile_pool(name="tmp", bufs=5) as tmp_pool,
            tc.tile_pool(name="outp", bufs=2) as outp,
        ):
            # resident inputs
            kh_sb = [None] * ND
            qh_sb = [None] * ND
            v_sb = [None] * H
            for h in range(ND):
                kt = consts.tile([P, S], f16, name=f"kh_sb{h}")
                nc.sync.dma_start(out=kt, in_=kh_d[h])
                kh_sb[h] = kt
                qt = consts.tile([P, S_LOC], f16, name=f"qh_sb{h}")
                nc.sync.dma_start(out=qt, in_=qh_d[h])
                qh_sb[h] = qt
            for h in range(H):
                vt = consts.tile([P, NTT, D], f16, name=f"v_sb{h}")
                nc.sync.dma_start(out=vt, in_=v_d[h])
                v_sb[h] = vt

            # Prime slot 7 of every exp buffer with 1.0: the head-sum tree
            # then adds the softmax "+1" for free, and the slot is never
            # overwritten (exps write slots 0-6, mul writes 0:7).
            primed = []
            for _ in range(EXP_BUFS):
                t = exp_pool.tile([P, H, SBLK], f16, name="exp_sb")
                nc.gpsimd.memset(t[:, 7, :], 1.0)
                primed.append(t)

            AV_LAG = 3

            def emit_mul(tt_, exp_tile, rcp_tile):
                # normalize slots 0:7 IN PLACE with r broadcast over heads
                rcp_b = bass.AP(
                    tensor=rcp_tile.tensor,
                    offset=rcp_tile.offset,
                    ap=[rcp_tile.ap[0], [0, ND], rcp_tile.ap[1]],
                )
                nc.vector.tensor_mul(
                    exp_tile[:, 0:ND, :], exp_tile[:, 0:ND, :], rcp_b
                )

            def emit_av_pair(oacc4, tt_, exp_tile, rcp_tile, pr):
                for j in range(2):
                    h = 2 * pr + j
                    rhs = rcp_tile if h == 7 else exp_tile[:, h, :]
                    nc.tensor.matmul(
                        out=oacc4[:, pr, :][j * D : (j + 1) * D, :],
                        lhsT=v_sb[h][:, tt_, :],
                        rhs=rhs,
                        start=(tt_ == 0),
                        stop=(tt_ == NTT - 1),
                    )

            AluAdd = mybir.AluOpType.add

            def emit_rsum_mul(item, pending):
                t_, e_, z2_, r_ = item
                nc.vector._custom_dve(
                    _RSUM, out=r_, in0=z2_[:, 0, :], in1=z2_[:, 1, :],
                    s0=-0.23549792, s1=2.0017324,
                )
                emit_mul(t_, e_, r_)
                pending.append((t_, e_, r_))

            for sb in range(NSB):
                oacc4 = oacc_pool.tile([P, NPAIR, SBLK], f32, name="oacc")
                pend_tree = []  # (tt, exp_sb, z2, rcp) awaiting rsum+mul
                pending = []    # (tt, exp_sb, rcp) mul done, awaiting AV
                for tt in range(NTT):
                    exp_sb = exp_pool.tile([P, H, SBLK], f16, name="exp_sb")
                    av = None
                    if len(pending) > AV_LAG - 2:
                        av = pending.pop(0)
                    # delta-head groups: (0,1), (2,3), (4,5), (6,)
                    for g in range(4):
                        heads = [2 * g, 2 * g + 1] if g < 3 else [6]
                        ps = score_pool.tile([P, 2, SBLK], f32, name="score")
                        for j, h in enumerate(heads):
                            nc.tensor.matmul(
                                out=ps[:, j, :],
                                lhsT=kh_sb[h][:, tt * P : (tt + 1) * P],
                                rhs=qh_sb[h][:, sb * SBLK : (sb + 1) * SBLK],
                                start=True,
                                stop=True,
                            )
                        if av is not None:
                            emit_av_pair(oacc4, av[0], av[1], av[2], g)
                        nc.scalar.activation(
                            out=exp_sb[:, heads[0] : heads[-1] + 1, :],
                            in_=ps[:, 0 : len(heads), :],
                            func=Exp,
                            scale=SCALE,
                        )
                    # head-sum tree: slot 7 == 1.0 supplies the softmax +1.
                    # z4 split in halves so the first starts after exp group
                    # g1; the z2 level rides the idle DMA engines (software
                    # DGE copy + accumulate), and rsum+mul lag one iteration
                    # so the DMA latency is hidden.
                    z4a = tmp_pool.tile([P, 2, SBLK], f16, name="z4a")
                    nc.vector.tensor_add(z4a, exp_sb[:, 0:2, :], exp_sb[:, 2:4, :])
                    z2 = tmp_pool.tile([P, 2, SBLK], f16, name="z2")
                    nc.gpsimd.dma_start(out=z2, in_=z4a)
                    if pend_tree:
                        emit_rsum_mul(pend_tree.pop(0), pending)
                    z4b = tmp_pool.tile([P, 2, SBLK], f16, name="z4b")
                    nc.vector.tensor_add(z4b, exp_sb[:, 4:6, :], exp_sb[:, 6:8, :])
                    nc.gpsimd.dma_start(out=z2, in_=z4b, accum_op=AluAdd)
                    rcp = tmp_pool.tile([P, SBLK], f16, name="rcp")
                    pend_tree.append((tt, exp_sb, z2, rcp))
                for item in pend_tree:
                    emit_rsum_mul(item, pending)
                pend_tree = []
                for t_, e_, r_ in pending:
                    for pr in range(NPAIR):
                        emit_av_pair(oacc4, t_, e_, r_, pr)
                pending = []
                ot = outp.tile([P, NPAIR, SBLK], f32, name="ot")
                # drain on ScalarE: it has headroom vs the DVE
                nc.scalar.copy(out=ot, in_=oacc4)
                nc.sync.dma_start(
                    out=out_d[:, :, sb * SBLK : (sb + 1) * SBLK].rearrange(
                        "p d s -> d p s"
                    ),
                    in_=ot,
                )

    nc.compile()
    return nc


def _get_nc():
    with _lock:
        if _cache["nc"] is None:
            _cache["nc"] = _build()
        return _cache["nc"]


def _prep_inputs(Q, K, V):
    Q = np.asarray(Q, dtype=np.float32)
    K = np.asarray(K, dtype=np.float32)
    V = np.asarray(V, dtype=np.float32)
    qt = np.ascontiguousarray(Q.transpose(0, 1, 3, 2)).astype(np.float16)  # [B,H,D,S]
    kb = K.astype(np.float16)  # [B,H,D,S]
    # stacked delta operands: KH[b,h] = [K_h; -K_7], QH[b,h] = [Q_h^T; Q_7^T]
    khat = np.empty((B, ND, P, S), dtype=np.float16)
    qhat = np.empty((B, ND, P, S), dtype=np.float16)
    for h in range(ND):
        khat[:, h, 0:D] = kb[:, h]
        khat[:, h, D:P] = -kb[:, 7]
        qhat[:, h, 0:D] = qt[:, h]
        qhat[:, h, D:P] = qt[:, 7]
    vp = np.ascontiguousarray(
        V.reshape(B, H, NTT, P, D).transpose(0, 1, 3, 2, 4)
    ).astype(np.float16)
    in_maps = []
    for c in range(N_CORES):
        b, sc = divmod(c, S_CHUNKS)
        in_maps.append(
            {
                "QH": np.ascontiguousarray(
                    qhat[b, :, :, sc * S_LOC : (sc + 1) * S_LOC]
                ),
                "KH": khat[b],
                "V": vp[b],
            }
        )
    return in_maps


def _assemble(results):
    # The reference output is a RAW reshape of contiguous [B, H, S, d] to
    # [B, S, H*d] (torch .view quirk), NOT a head-transpose.
    o_full = np.empty((B, H, S, D), dtype=np.float32)
    for c in range(N_CORES):
        b, sc = divmod(c, S_CHUNKS)
        shard = results[c]["OUT"]  # [NPAIR, 128, S_LOC]
        o_full[b, :, sc * S_LOC : (sc + 1) * S_LOC, :] = (
            shard.reshape(NPAIR, 2, D, S_LOC).transpose(0, 1, 3, 2).reshape(
                H, S_LOC, D
            )
        )
    return o_full.reshape(B, S, HIDDEN)


def run(Q, K, V, trace=False, **run_kwargs):
    nc = _get_nc()
    in_maps = _prep_inputs(Q, K, V)
    res = run_bass_kernel_spmd(
        nc, in_maps, core_ids=list(range(N_CORES)), trace=trace, **run_kwargs
    )
    return _assemble(res.results), res


def kernel(Q, K, V):
    prev = os.environ.get("BASS_NEVER_TRACE")
    os.environ["BASS_NEVER_TRACE"] = "1"
    try:
        out, _ = run(Q, K, V, trace=False)
    finally:
        if prev is None:
            os.environ.pop("BASS_NEVER_TRACE", None)
        else:
            os.environ["BASS_NEVER_TRACE"] = prev
    return out


# revision 31
# speedup vs baseline: 1.1380x; 1.1380x over previous
"""Trainium2 Bass kernel for nn_Attention_20925080666453 (delta-softmax).

Computation (faithful to the torch module quirk):
    e = (Q @ K) / sqrt(512)            # [B,H,S,S]
    a = softmax(e, axis=1)             # softmax over the HEAD axis
    o = a @ V                          # [B,H,S,d]
    out = o.reshape(B, S, H*d)

Head-axis softmax is invariant to subtracting head 7's scores:
    delta_h = e_h - e_7 (h=0..6), delta_7 = 0
    a_h = exp(delta_h) / (1 + sum_{j<7} exp(delta_j)),  a_7 = r = 1/(1 + sum)
Each delta is ONE contraction-128 matmul with stacked operands
lhsT = [K_h; -K_7], rhs = [Q_h^T; Q_7^T]; head 7 needs no score matmul,
no exp, and no normalize-mul (its AV moving operand is r itself).

Sharding: 8 cores = batch (2) x query-chunk (4); no collectives.

Per-core pipeline per (s-block, t-tile) iteration:
  - 7 delta-score MMs -> PSUM [t=128, s=512], exp on ScalarE (scale fused,
    fp16 out, 3x FD-1024 + 1x FD-512 instructions).
  - exp_sb slot 7 is pre-primed to 1.0 (once per pool buffer), so the
    z4/z2 tree sums exp deltas AND the +1 in full-width 2x DVE adds.
  - fused custom DVE op: rcp = recip1(z2[0] + z2[1])  (bit-trick seed +
    one tuned Newton step, ~0.2% max rel err).
  - normalize-mul over slots 0:7 with rcp broadcast; AV matmuls lag 2
    iterations and are interleaved pairwise between score matmuls.
"""

import os
import sys
import threading

sys.path.insert(0, "/opt/trn_rl_repo")

import numpy as np

import concourse.bacc as bacc
import concourse.bass as bass
import concourse.mybir as mybir
import concourse.tile as tile
from concourse.bass_utils import run_bass_kernel_spmd

# Problem dims
B, H, S, D = 2, 8, 4096, 64
HIDDEN = H * D
SCALE = float(1.0 / np.sqrt(np.float32(HIDDEN)))

P = 128              # partitions
NPAIR = H // 2       # head pairs
ND = H - 1           # delta heads (vs reference head 7)
N_CORES = 8
S_CHUNKS = 4         # query chunks per batch
S_LOC = S // S_CHUNKS    # 1024 queries per core
SBLK = 512               # s-block (one PSUM bank of fp32)
NSB = S_LOC // SBLK      # 2
NTT = S // P             # 32 key tiles of 128

EXP_BUFS = 6

_cache = {"nc": None}
_lock = threading.Lock()


def _register_recip_sum_op():
    """Register a fused custom DVE op: out = recip1(in0 + in1)."""
    import concourse.dve_ops as dve_ops
    from concourse.dve_ops import DveOp, OPS, _SUB_OPCODE_FOR_NAME, CUSTOM_DVE_SPECS
    from concourse.dve_spec import Spec, Src0, Src1, Bin, AluOp, C0, C1, lower
    from concourse.dve_uop import DveOpSpec
    from concourse.dve_table_gen import dve_ver_for

    name = "RECIP_SUM1_ANT"
    for op in OPS:
        if op.name == name:
            return op
    x = Src0 + Src1
    not_x = Bin(AluOp.BITWISE_NOT, x, x)
    y0 = not_x * C0
    body = y0 * (C1 - x * y0)

    def ref(in0, in1, c0, c1, c2):
        xx = np.asarray(in0, np.float32) + np.asarray(in1, np.float32)
        nx = (~xx.view(np.int32)).view(np.float32)
        yy0 = nx * c0
        return yy0 * (c1 - xx * yy0)

    spec = Spec(body=body, reference=ref)
    _SUB_OPCODE_FOR_NAME.setdefault(name, max(_SUB_OPCODE_FOR_NAME.values()) + 1)
    ver = dve_ver_for("TRN2")
    uops = lower(spec, ver=ver)
    sha = DveOpSpec(
        name=name, opcode=_SUB_OPCODE_FOR_NAME[name], uops=uops, rd1_en=True
    ).sha(ver)
    op = DveOp(name, spec, subdim=False, uops_sha={ver: sha})
    OPS.append(op)
    CUSTOM_DVE_SPECS[name] = spec
    return op


def _build():
    global _RSUM
    _RSUM = _register_recip_sum_op()
    nc = bacc.Bacc(
        "TRN2",
        target_bir_lowering=False,
        debug=False,
        enable_asserts=True,
        num_devices=N_CORES,
    )
    f32 = mybir.dt.float32
    f16 = mybir.dt.float16

    qh_d = nc.dram_tensor("QH", [ND, P, S_LOC], f16, kind="ExternalInput").ap()
    kh_d = nc.dram_tensor("KH", [ND, P, S], f16, kind="ExternalInput").ap()
    v_d = nc.dram_tensor("V", [H, P, NTT, D], f16, kind="ExternalInput").ap()
    out_d = nc.dram_tensor("OUT", [NPAIR, P, S_LOC], f32, kind="ExternalOutput").ap()

    Exp = mybir.ActivationFunctionType.Exp

    with tile.TileContext(nc) as tc:
        with (
            tc.tile_pool(name="consts", bufs=1) as consts,
            tc.tile_pool(name="score", bufs=2, space="PSUM") as score_pool,
            tc.tile_pool(name="oaccp", bufs=1, space="PSUM") as oacc_pool,
            tc.tile_pool(name="expp", bufs=EXP_BUFS) as exp_pool,
            tc.tile_pool(name="tmp", bufs=5) as tmp_pool,
            tc.tile_pool(name="outp", bufs=2) as outp,
        ):
            # resident inputs
            kh_sb = [None] * ND
            qh_sb = [None] * ND
            v_sb = [None] * H
            for h in range(ND):
                kt = consts.tile([P, S], f16, name=f"kh_sb{h}")
                nc.sync.dma_start(out=kt, in_=kh_d[h])
                kh_sb[h] = kt
                qt = consts.tile([P, S_LOC], f16, name=f"qh_sb{h}")
                nc.sync.dma_start(out=qt, in_=qh_d[h])
                qh_sb[h] = qt
            for h in range(H):
                vt = consts.tile([P, NTT, D], f16, name=f"v_sb{h}")
                nc.sync.dma_start(out=vt, in_=v_d[h])
                v_sb[h] = vt

            # Prime slot 7 of every exp buffer with 1.0: the head-sum tree
            # then adds the softmax "+1" for free, and the slot is never
            # overwritten (exps write slots 0-6, mul writes 0:7).
            primed = []
            for _ in range(EXP_BUFS):
                t = exp_pool.tile([P, H, SBLK], f16, name="exp_sb")
                nc.gpsimd.memset(t[:, 7, :], 1.0)
                primed.append(t)

            AV_LAG = 3

            def emit_mul(tt_, exp_tile, rcp_tile):
                # normalize slots 0:7 IN PLACE with r broadcast over heads
                rcp_b = bass.AP(
                    tensor=rcp_tile.tensor,
                    offset=rcp_tile.offset,
                    ap=[rcp_tile.ap[0], [0, ND], rcp_tile.ap[1]],
                )
                nc.vector.tensor_mul(
                    exp_tile[:, 0:ND, :], exp_tile[:, 0:ND, :], rcp_b
                )

            def emit_av_pair(oacc4, tt_, exp_tile, rcp_tile, pr):
                for j in range(2):
                    h = 2 * pr + j
                    rhs = rcp_tile if h == 7 else exp_tile[:, h, :]
                    nc.tensor.matmul(
                        out=oacc4[:, pr, :][j * D : (j + 1) * D, :],
                        lhsT=v_sb[h][:, tt_, :],
                        rhs=rhs,
                        start=(tt_ == 0),
                        stop=(tt_ == NTT - 1),
                    )

            for sb in range(NSB):
                oacc4 = oacc_pool.tile([P, NPAIR, SBLK], f32, name="oacc")
                pend_mul = []  # (tt, exp_sb, rcp) awaiting normalize-mul
                pending = []   # (tt, exp_sb, rcp) mul done, awaiting AV
                for tt in range(NTT):
                    exp_sb = exp_pool.tile([P, H, SBLK], f16, name="exp_sb")
                    av = None
                    if len(pending) > AV_LAG - 2:
                        av = pending.pop(0)
                    # delta-head groups: (0,1), (2,3), (4,5), (6,)
                    for g in range(4):
                        heads = [2 * g, 2 * g + 1] if g < 3 else [6]
                        ps = score_pool.tile([P, 2, SBLK], f32, name="score")
                        for j, h in enumerate(heads):
                            nc.tensor.matmul(
                                out=ps[:, j, :],
                                lhsT=kh_sb[h][:, tt * P : (tt + 1) * P],
                                rhs=qh_sb[h][:, sb * SBLK : (sb + 1) * SBLK],
                                start=True,
                                stop=True,
                            )
                        if av is not None:
                            emit_av_pair(oacc4, av[0], av[1], av[2], g)
                        nc.scalar.activation(
                            out=exp_sb[:, heads[0] : heads[-1] + 1, :],
                            in_=ps[:, 0 : len(heads), :],
                            func=Exp,
                            scale=SCALE,
                        )
                    # head-sum tree: slot 7 == 1.0 supplies the softmax +1.
                    # z4 split in halves: z4a starts after exp group g1 (not
                    # g3), and the mul of the previous iteration fills the
                    # DVE until the last exp lands.
                    z4a = tmp_pool.tile([P, 2, SBLK], f16, name="z4a")
                    nc.vector.tensor_add(z4a, exp_sb[:, 0:2, :], exp_sb[:, 2:4, :])
                    if pend_mul:
                        t_, e_, r_ = pend_mul.pop(0)
                        emit_mul(t_, e_, r_)
                        pending.append((t_, e_, r_))
                    z4b = tmp_pool.tile([P, 2, SBLK], f16, name="z4b")
                    nc.vector.tensor_add(z4b, exp_sb[:, 4:6, :], exp_sb[:, 6:8, :])
                    z2 = tmp_pool.tile([P, 2, SBLK], f16, name="z2")
                    nc.vector.tensor_add(z2, z4a, z4b)
                    rcp = tmp_pool.tile([P, SBLK], f16, name="rcp")
                    nc.vector._custom_dve(
                        _RSUM, out=rcp, in0=z2[:, 0, :], in1=z2[:, 1, :],
                        s0=-0.23549792, s1=2.0017324,
                    )
                    pend_mul.append((tt, exp_sb, rcp))
                for t_, e_, r_ in pend_mul:
                    emit_mul(t_, e_, r_)
                    pending.append((t_, e_, r_))
                pend_mul = []
                for t_, e_, r_ in pending:
                    for pr in range(NPAIR):
                        emit_av_pair(oacc4, t_, e_, r_, pr)
                pending = []
                ot = outp.tile([P, NPAIR, SBLK], f32, name="ot")
                # drain on ScalarE: it has headroom vs the DVE
                nc.scalar.copy(out=ot, in_=oacc4)
                nc.sync.dma_start(
                    out=out_d[:, :, sb * SBLK : (sb + 1) * SBLK].rearrange(
                        "p d s -> d p s"
                    ),
                    in_=ot,
                )

    nc.compile()
    return nc


def _get_nc():
    with _lock:
        if _cache["nc"] is None:
            _cache["nc"] = _build()
        return _cache["nc"]


def _prep_inputs(Q, K, V):
    Q = np.asarray(Q, dtype=np.float32)
    K = np.asarray(K, dtype=np.float32)
    V = np.asarray(V, dtype=np.float32)
    qt = np.ascontiguousarray(Q.transpose(0, 1, 3, 2)).astype(np.float16)  # [B,H,D,S]
    kb = K.astype(np.float16)  # [B,H,D,S]
    # stacked delta operands: KH[b,h] = [K_h; -K_7], QH[b,h] = [Q_h^T; Q_7^T]
    khat = np.empty((B, ND, P, S), dtype=np.float16)
    qhat = np.empty((B, ND, P, S), dtype=np.float16)
    for h in range(ND):
        khat[:, h, 0:D] = kb[:, h]
        khat[:, h, D:P] = -kb[:, 7]
        qhat[:, h, 0:D] = qt[:, h]
        qhat[:, h, D:P] = qt[:, 7]
    vp = np.ascontiguousarray(
        V.reshape(B, H, NTT, P, D).transpose(0, 1, 3, 2, 4)
    ).astype(np.float16)
    in_maps = []
    for c in range(N_CORES):
        b, sc = divmod(c, S_CHUNKS)
        in_maps.append(
            {
                "QH": np.ascontiguousarray(
                    qhat[b, :, :, sc * S_LOC : (sc + 1) * S_LOC]
                ),
                "KH": khat[b],
                "V": vp[b],
            }
        )
    return in_maps


def _assemble(results):
    # The reference output is a RAW reshape of contiguous [B, H, S, d] to
    # [B, S, H*d] (torch .view quirk), NOT a head-transpose.
    o_full = np.empty((B, H, S, D), dtype=np.float32)
    for c in range(N_CORES):
        b, sc = divmod(c, S_CHUNKS)
        shard = results[c]["OUT"]  # [NPAIR, 128, S_LOC]
        o_full[b, :, sc * S_LOC : (sc + 1) * S_LOC, :] = (
            shard.reshape(NPAIR, 2, D, S_LOC).transpose(0, 1, 3, 2).reshape(
                H, S_LOC, D
            )
        )
    return o_full.reshape(B, S, HIDDEN)


def run(Q, K, V, trace=False, **run_kwargs):
    nc = _get_nc()
    in_maps = _prep_inputs(Q, K, V)
    res = run_bass_kernel_spmd(
        nc, in_maps, core_ids=list(range(N_CORES)), trace=trace, **run_kwargs
    )
    return _assemble(res.results), res


def kernel(Q, K, V):
    prev = os.environ.get("BASS_NEVER_TRACE")
    os.environ["BASS_NEVER_TRACE"] = "1"
    try:
        out, _ = run(Q, K, V, trace=False)
    finally:
        if prev is None:
            os.environ.pop("BASS_NEVER_TRACE", None)
        else:
            os.environ["BASS_NEVER_TRACE"] = prev
    return out


# revision 33
# speedup vs baseline: 1.1714x; 1.0294x over previous
"""Trainium2 Bass kernel for nn_Attention_20925080666453 (delta-softmax).

Computation (faithful to the torch module quirk):
    e = (Q @ K) / sqrt(512)            # [B,H,S,S]
    a = softmax(e, axis=1)             # softmax over the HEAD axis
    o = a @ V                          # [B,H,S,d]
    out = o.reshape(B, S, H*d)

Head-axis softmax is invariant to subtracting head 7's scores:
    delta_h = e_h - e_7 (h=0..6), delta_7 = 0
    a_h = exp(delta_h) / (1 + sum_{j<7} exp(delta_j)),  a_7 = r = 1/(1 + sum)
Each delta is ONE contraction-128 matmul with stacked operands
lhsT = [K_h; -K_7], rhs = [Q_h^T; Q_7^T]; head 7 needs no score matmul,
no exp, and no normalize-mul (its AV moving operand is r itself).

Sharding: 8 cores = batch (2) x query-chunk (4); no collectives.

Per-core pipeline per (s-block, t-tile) iteration:
  - 7 delta-score MMs -> PSUM [t=128, s=512], exp on ScalarE (scale fused,
    fp16 out, 3x FD-1024 + 1x FD-512 instructions).
  - exp_sb slot 7 is pre-primed to 1.0 (once per pool buffer), so the
    z4/z2 tree sums exp deltas AND the +1 in full-width 2x DVE adds.
  - fused custom DVE op: rcp = recip1(z2[0] + z2[1])  (bit-trick seed +
    one tuned Newton step, ~0.2% max rel err).
  - normalize-mul over slots 0:7 with rcp broadcast; AV matmuls lag 2
    iterations and are interleaved pairwise between score matmuls.
"""

import os
import sys
import threading

sys.path.insert(0, "/opt/trn_rl_repo")

import numpy as np

import concourse.bacc as bacc
import concourse.bass as bass
import concourse.mybir as mybir
import concourse.tile as tile
from concourse.bass_utils import run_bass_kernel_spmd

# Problem dims
B, H, S, D = 2, 8, 4096, 64
HIDDEN = H * D
SCALE = float(1.0 / np.sqrt(np.float32(HIDDEN)))

P = 128              # partitions
NPAIR = H // 2       # head pairs
ND = H - 1           # delta heads (vs reference head 7)
N_CORES = 8
S_CHUNKS = 4         # query chunks per batch
S_LOC = S // S_CHUNKS    # 1024 queries per core
SBLK = 512               # s-block (one PSUM bank of fp32)
NSB = S_LOC // SBLK      # 2
NTT = S // P             # 32 key tiles of 128

EXP_BUFS = 6

_cache = {"nc": None}
_lock = threading.Lock()


def _register_recip_sum_op():
    """Register a fused custom DVE op: out = recip1(in0 + in1)."""
    import concourse.dve_ops as dve_ops
    from concourse.dve_ops import DveOp, OPS, _SUB_OPCODE_FOR_NAME, CUSTOM_DVE_SPECS
    from concourse.dve_spec import Spec, Src0, Src1, Bin, AluOp, C0, C1, lower
    from concourse.dve_uop import DveOpSpec
    from concourse.dve_table_gen import dve_ver_for

    name = "RECIP_SUM1_ANT"
    for op in OPS:
        if op.name == name:
            return op
    x = Src0 + Src1
    not_x = Bin(AluOp.BITWISE_NOT, x, x)
    y0 = not_x * C0
    body = y0 * (C1 - x * y0)

    def ref(in0, in1, c0, c1, c2):
        xx = np.asarray(in0, np.float32) + np.asarray(in1, np.float32)
        nx = (~xx.view(np.int32)).view(np.float32)
        yy0 = nx * c0
        return yy0 * (c1 - xx * yy0)

    spec = Spec(body=body, reference=ref)
    _SUB_OPCODE_FOR_NAME.setdefault(name, max(_SUB_OPCODE_FOR_NAME.values()) + 1)
    ver = dve_ver_for("TRN2")
    uops = lower(spec, ver=ver)
    sha = DveOpSpec(
        name=name, opcode=_SUB_OPCODE_FOR_NAME[name], uops=uops, rd1_en=True
    ).sha(ver)
    op = DveOp(name, spec, subdim=False, uops_sha={ver: sha})
    OPS.append(op)
    CUSTOM_DVE_SPECS[name] = spec
    return op


def _build():
    global _RSUM
    _RSUM = _register_recip_sum_op()
    nc = bacc.Bacc(
        "TRN2",
        target_bir_lowering=False,
        debug=False,
        enable_asserts=True,
        num_devices=N_CORES,
    )
    f32 = mybir.dt.float32
    f16 = mybir.dt.float16

    qh_d = nc.dram_tensor("QH", [ND, P, S_LOC], f16, kind="ExternalInput").ap()
    kh_d = nc.dram_tensor("KH", [ND, P, S], f16, kind="ExternalInput").ap()
    v_d = nc.dram_tensor("V", [H, P, NTT, D], f16, kind="ExternalInput").ap()
    out_d = nc.dram_tensor("OUT", [NPAIR, P, S_LOC], f32, kind="ExternalOutput").ap()

    Exp = mybir.ActivationFunctionType.Exp

    with tile.TileContext(nc) as tc:
        with (
            tc.tile_pool(name="consts", bufs=1) as consts,
            tc.tile_pool(name="score", bufs=2, space="PSUM") as score_pool,
            tc.tile_pool(name="oaccp", bufs=1, space="PSUM") as oacc_pool,
            tc.tile_pool(name="expp", bufs=EXP_BUFS) as exp_pool,
            tc.tile_pool(name="tmp", bufs=5) as tmp_pool,
            tc.tile_pool(name="outp", bufs=2) as outp,
        ):
            # resident inputs, loaded in t-chunks so the first score matmuls
            # only wait for the first chunk instead of the whole 11 MB
            kh_sb = [None] * ND
            qh_sb = [None] * ND
            v_sb = [None] * H
            for h in range(ND):
                kh_sb[h] = consts.tile([P, S], f16, name=f"kh_sb{h}")
                qh_sb[h] = consts.tile([P, S_LOC], f16, name=f"qh_sb{h}")
            for h in range(H):
                v_sb[h] = consts.tile([P, NTT, D], f16, name=f"v_sb{h}")
            for h in range(ND):
                nc.sync.dma_start(
                    out=qh_sb[h][:, 0:SBLK], in_=qh_d[h][:, 0:SBLK]
                )
            NCH = 4
            CS = S // NCH       # kh chunk: 8 t-tiles
            CT = NTT // NCH
            for c in range(NCH):
                for h in range(ND):
                    nc.sync.dma_start(
                        out=kh_sb[h][:, c * CS : (c + 1) * CS],
                        in_=kh_d[h][:, c * CS : (c + 1) * CS],
                    )
                for h in range(H):
                    nc.sync.dma_start(
                        out=v_sb[h][:, c * CT : (c + 1) * CT, :],
                        in_=v_d[h][:, c * CT : (c + 1) * CT, :],
                    )
                if c == 0:
                    for h in range(ND):
                        nc.sync.dma_start(
                            out=qh_sb[h][:, SBLK:S_LOC],
                            in_=qh_d[h][:, SBLK:S_LOC],
                        )

            # Prime slot 7 of every exp buffer with 1.0: the head-sum tree
            # then adds the softmax "+1" for free, and the slot is never
            # overwritten (exps write slots 0-6, mul writes 0:7).
            primed = []
            for _ in range(EXP_BUFS):
                t = exp_pool.tile([P, H, SBLK], f16, name="exp_sb")
                nc.gpsimd.memset(t[:, 7, :], 1.0)
                primed.append(t)

            AV_LAG = 3

            def emit_mul(tt_, exp_tile, rcp_tile):
                # normalize slots 0:7 IN PLACE with r broadcast over heads
                rcp_b = bass.AP(
                    tensor=rcp_tile.tensor,
                    offset=rcp_tile.offset,
                    ap=[rcp_tile.ap[0], [0, ND], rcp_tile.ap[1]],
                )
                nc.vector.tensor_mul(
                    exp_tile[:, 0:ND, :], exp_tile[:, 0:ND, :], rcp_b
                )

            def emit_av_pair(oacc4, tt_, exp_tile, rcp_tile, pr):
                for j in range(2):
                    h = 2 * pr + j
                    rhs = rcp_tile if h == 7 else exp_tile[:, h, :]
                    nc.tensor.matmul(
                        out=oacc4[:, pr, :][j * D : (j + 1) * D, :],
                        lhsT=v_sb[h][:, tt_, :],
                        rhs=rhs,
                        start=(tt_ == 0),
                        stop=(tt_ == NTT - 1),
                    )

            for sb in range(NSB):
                oacc4 = oacc_pool.tile([P, NPAIR, SBLK], f32, name="oacc")
                pend_mul = []  # (tt, exp_sb, rcp) awaiting normalize-mul
                pending = []   # (tt, exp_sb, rcp) mul done, awaiting AV
                for tt in range(NTT):
                    exp_sb = exp_pool.tile([P, H, SBLK], f16, name="exp_sb")
                    av = None
                    if len(pending) > AV_LAG - 2:
                        av = pending.pop(0)
                    # delta-head groups: (0,1), (2,3), (4,5), (6,)
                    for g in range(4):
                        heads = [2 * g, 2 * g + 1] if g < 3 else [6]
                        ps = score_pool.tile([P, 2, SBLK], f32, name="score")
                        for j, h in enumerate(heads):
                            nc.tensor.matmul(
                                out=ps[:, j, :],
                                lhsT=kh_sb[h][:, tt * P : (tt + 1) * P],
                                rhs=qh_sb[h][:, sb * SBLK : (sb + 1) * SBLK],
                                start=True,
                                stop=True,
                            )
                        if av is not None:
                            emit_av_pair(oacc4, av[0], av[1], av[2], g)
                        nc.scalar.activation(
                            out=exp_sb[:, heads[0] : heads[-1] + 1, :],
                            in_=ps[:, 0 : len(heads), :],
                            func=Exp,
                            scale=SCALE,
                        )
                    # head-sum tree: slot 7 == 1.0 supplies the softmax +1.
                    # z4 split in halves: z4a starts after exp group g1 (not
                    # g3), and the mul of the previous iteration fills the
                    # DVE until the last exp lands.
                    z4a = tmp_pool.tile([P, 2, SBLK], f16, name="z4a")
                    nc.vector.tensor_add(z4a, exp_sb[:, 0:2, :], exp_sb[:, 2:4, :])
                    if pend_mul:
                        t_, e_, r_ = pend_mul.pop(0)
                        emit_mul(t_, e_, r_)
                        pending.append((t_, e_, r_))
                    z4b = tmp_pool.tile([P, 2, SBLK], f16, name="z4b")
                    nc.vector.tensor_add(z4b, exp_sb[:, 4:6, :], exp_sb[:, 6:8, :])
                    z2 = tmp_pool.tile([P, 2, SBLK], f16, name="z2")
                    nc.vector.tensor_add(z2, z4a, z4b)
                    rcp = tmp_pool.tile([P, SBLK], f16, name="rcp")
                    nc.vector._custom_dve(
                        _RSUM, out=rcp, in0=z2[:, 0, :], in1=z2[:, 1, :],
                        s0=-0.23549792, s1=2.0017324,
                    )
                    pend_mul.append((tt, exp_sb, rcp))
                for t_, e_, r_ in pend_mul:
                    emit_mul(t_, e_, r_)
                    pending.append((t_, e_, r_))
                pend_mul = []
                for t_, e_, r_ in pending:
                    for pr in range(NPAIR):
                        emit_av_pair(oacc4, t_, e_, r_, pr)
                pending = []
                # drain per pair on ScalarE so output DMAs overlap the tail
                ot = outp.tile([P, NPAIR, SBLK], f32, name="ot")
                for pr in range(NPAIR):
                    nc.scalar.copy(out=ot[:, pr, :], in_=oacc4[:, pr, :])
                    nc.sync.dma_start(
                        out=out_d[pr, :, sb * SBLK : (sb + 1) * SBLK],
                        in_=ot[:, pr, :],
                    )

    nc.compile()
    return nc


def _get_nc():
    with _lock:
        if _cache["nc"] is None:
            _cache["nc"] = _build()
        return _cache["nc"]


def _prep_inputs(Q, K, V):
    Q = np.asarray(Q, dtype=np.float32)
    K = np.asarray(K, dtype=np.float32)
    V = np.asarray(V, dtype=np.float32)
    qt = np.ascontiguousarray(Q.transpose(0, 1, 3, 2)).astype(np.float16)  # [B,H,D,S]
    kb = K.astype(np.float16)  # [B,H,D,S]
    # stacked delta operands: KH[b,h] = [K_h; -K_7], QH[b,h] = [Q_h^T; Q_7^T]
    khat = np.empty((B, ND, P, S), dtype=np.float16)
    qhat = np.empty((B, ND, P, S), dtype=np.float16)
    for h in range(ND):
        khat[:, h, 0:D] = kb[:, h]
        khat[:, h, D:P] = -kb[:, 7]
        qhat[:, h, 0:D] = qt[:, h]
        qhat[:, h, D:P] = qt[:, 7]
    vp = np.ascontiguousarray(
        V.reshape(B, H, NTT, P, D).transpose(0, 1, 3, 2, 4)
    ).astype(np.float16)
    in_maps = []
    for c in range(N_CORES):
        b, sc = divmod(c, S_CHUNKS)
        in_maps.append(
            {
                "QH": np.ascontiguousarray(
                    qhat[b, :, :, sc * S_LOC : (sc + 1) * S_LOC]
                ),
                "KH": khat[b],
                "V": vp[b],
            }
        )
    return in_maps


def _assemble(results):
    # The reference output is a RAW reshape of contiguous [B, H, S, d] to
    # [B, S, H*d] (torch .view quirk), NOT a head-transpose.
    o_full = np.empty((B, H, S, D), dtype=np.float32)
    for c in range(N_CORES):
        b, sc = divmod(c, S_CHUNKS)
        shard = results[c]["OUT"]  # [NPAIR, 128, S_LOC]
        o_full[b, :, sc * S_LOC : (sc + 1) * S_LOC, :] = (
            shard.reshape(NPAIR, 2, D, S_LOC).transpose(0, 1, 3, 2).reshape(
                H, S_LOC, D
            )
        )
    return o_full.reshape(B, S, HIDDEN)


def run(Q, K, V, trace=False, **run_kwargs):
    nc = _get_nc()
    in_maps = _prep_inputs(Q, K, V)
    res = run_bass_kernel_spmd(
        nc, in_maps, core_ids=list(range(N_CORES)), trace=trace, **run_kwargs
    )
    return _assemble(res.results), res


def kernel(Q, K, V):
    prev = os.environ.get("BASS_NEVER_TRACE")
    os.environ["BASS_NEVER_TRACE"] = "1"
    try:
        out, _ = run(Q, K, V, trace=False)
    finally:
        if prev is None:
            os.environ.pop("BASS_NEVER_TRACE", None)
        else:
            os.environ["BASS_NEVER_TRACE"] = prev
    return out


# revision 35
# speedup vs baseline: 1.1893x; 1.0153x over previous
"""Trainium2 Bass kernel for nn_Attention_20925080666453 (delta-softmax).

Computation (faithful to the torch module quirk):
    e = (Q @ K) / sqrt(512)            # [B,H,S,S]
    a = softmax(e, axis=1)             # softmax over the HEAD axis
    o = a @ V                          # [B,H,S,d]
    out = o.reshape(B, S, H*d)

Head-axis softmax is invariant to subtracting head 7's scores:
    delta_h = e_h - e_7 (h=0..6), delta_7 = 0
    a_h = exp(delta_h) / (1 + sum_{j<7} exp(delta_j)),  a_7 = r = 1/(1 + sum)
Each delta is ONE contraction-128 matmul with stacked operands
lhsT = [K_h; -K_7], rhs = [Q_h^T; Q_7^T]; head 7 needs no score matmul,
no exp, and no normalize-mul (its AV moving operand is r itself).

Sharding: 8 cores = batch (2) x query-chunk (4); no collectives.

Per-core pipeline per (s-block, t-tile) iteration:
  - 7 delta-score MMs -> PSUM [t=128, s=512], exp on ScalarE (scale fused,
    fp16 out, 3x FD-1024 + 1x FD-512 instructions).
  - exp_sb slot 7 is pre-primed to 1.0 (once per pool buffer), so the
    z4/z2 tree sums exp deltas AND the +1 in full-width 2x DVE adds.
  - fused custom DVE op: rcp = recip1(z2[0] + z2[1])  (bit-trick seed +
    one tuned Newton step, ~0.2% max rel err).
  - normalize-mul over slots 0:7 with rcp broadcast; AV matmuls lag 2
    iterations and are interleaved pairwise between score matmuls.
"""

import os
import sys
import threading

sys.path.insert(0, "/opt/trn_rl_repo")

import numpy as np

import concourse.bacc as bacc
import concourse.bass as bass
import concourse.mybir as mybir
import concourse.tile as tile
from concourse.bass_utils import run_bass_kernel_spmd

# Problem dims
B, H, S, D = 2, 8, 4096, 64
HIDDEN = H * D
SCALE = float(1.0 / np.sqrt(np.float32(HIDDEN)))

P = 128              # partitions
NPAIR = H // 2       # head pairs
ND = H - 1           # delta heads (vs reference head 7)
N_CORES = 8
S_CHUNKS = 4         # query chunks per batch
S_LOC = S // S_CHUNKS    # 1024 queries per core
SBLK = 512               # s-block (one PSUM bank of fp32)
NSB = S_LOC // SBLK      # 2
NTT = S // P             # 32 key tiles of 128

EXP_BUFS = 6

_cache = {"nc": None}
_lock = threading.Lock()


def _register_recip_sum_op():
    """Register a fused custom DVE op: out = recip1(in0 + in1)."""
    import concourse.dve_ops as dve_ops
    from concourse.dve_ops import DveOp, OPS, _SUB_OPCODE_FOR_NAME, CUSTOM_DVE_SPECS
    from concourse.dve_spec import Spec, Src0, Src1, Bin, AluOp, C0, C1, lower
    from concourse.dve_uop import DveOpSpec
    from concourse.dve_table_gen import dve_ver_for

    name = "RECIP_SUM1_ANT"
    for op in OPS:
        if op.name == name:
            return op
    x = Src0 + Src1
    not_x = Bin(AluOp.BITWISE_NOT, x, x)
    y0 = not_x * C0
    body = y0 * (C1 - x * y0)

    def ref(in0, in1, c0, c1, c2):
        xx = np.asarray(in0, np.float32) + np.asarray(in1, np.float32)
        nx = (~xx.view(np.int32)).view(np.float32)
        yy0 = nx * c0
        return yy0 * (c1 - xx * yy0)

    spec = Spec(body=body, reference=ref)
    _SUB_OPCODE_FOR_NAME.setdefault(name, max(_SUB_OPCODE_FOR_NAME.values()) + 1)
    ver = dve_ver_for("TRN2")
    uops = lower(spec, ver=ver)
    sha = DveOpSpec(
        name=name, opcode=_SUB_OPCODE_FOR_NAME[name], uops=uops, rd1_en=True
    ).sha(ver)
    op = DveOp(name, spec, subdim=False, uops_sha={ver: sha})
    OPS.append(op)
    CUSTOM_DVE_SPECS[name] = spec
    return op


def _build():
    global _RSUM
    _RSUM = _register_recip_sum_op()
    nc = bacc.Bacc(
        "TRN2",
        target_bir_lowering=False,
        debug=False,
        enable_asserts=True,
        num_devices=N_CORES,
    )
    f32 = mybir.dt.float32
    f16 = mybir.dt.float16

    qh_d = nc.dram_tensor("QH", [ND, P, S_LOC], f16, kind="ExternalInput").ap()
    kh_d = nc.dram_tensor("KH", [ND, P, S], f16, kind="ExternalInput").ap()
    v_d = nc.dram_tensor("V", [H, P, NTT, D], f16, kind="ExternalInput").ap()
    out_d = nc.dram_tensor("OUT", [NPAIR, P, S_LOC], f32, kind="ExternalOutput").ap()

    Exp = mybir.ActivationFunctionType.Exp

    with tile.TileContext(nc) as tc:
        with (
            tc.tile_pool(name="consts", bufs=1) as consts,
            tc.tile_pool(name="score", bufs=2, space="PSUM") as score_pool,
            tc.tile_pool(name="oaccp", bufs=1, space="PSUM") as oacc_pool,
            tc.tile_pool(name="expp", bufs=EXP_BUFS) as exp_pool,
            tc.tile_pool(name="tmp", bufs=5) as tmp_pool,
            tc.tile_pool(name="outp", bufs=2) as outp,
        ):
            # resident inputs, loaded in t-chunks so the first score matmuls
            # only wait for the first chunk instead of the whole 11 MB
            kh_sb = [None] * ND
            qh_sb = [None] * ND
            v_sb = [None] * H
            for h in range(ND):
                kh_sb[h] = consts.tile([P, S], f16, name=f"kh_sb{h}")
                qh_sb[h] = consts.tile([P, S_LOC], f16, name=f"qh_sb{h}")
            for h in range(H):
                v_sb[h] = consts.tile([P, NTT, D], f16, name=f"v_sb{h}")
            # First small chunk (t-tiles 0-3) issued round-robin across the
            # idle engine queues so the first score MMs start within ~3us;
            # the remainder streams in behind on the sync queue.
            qs = [nc.sync, nc.scalar, nc.gpsimd]
            F = 4 * P  # first kh chunk: t-tiles 0-3
            for h in range(ND):
                qs[h % 3].dma_start(
                    out=kh_sb[h][:, 0:F], in_=kh_d[h][:, 0:F]
                )
            for h in range(ND):
                qs[(h + 1) % 3].dma_start(
                    out=qh_sb[h][:, 0:SBLK], in_=qh_d[h][:, 0:SBLK]
                )
            for h in range(H):
                qs[(h + 2) % 3].dma_start(
                    out=v_sb[h][:, 0:8, :], in_=v_d[h][:, 0:8, :]
                )
            for h in range(ND):
                nc.sync.dma_start(out=kh_sb[h][:, F:S], in_=kh_d[h][:, F:S])
            for h in range(H):
                nc.sync.dma_start(
                    out=v_sb[h][:, 8:NTT, :], in_=v_d[h][:, 8:NTT, :]
                )
            for h in range(ND):
                nc.sync.dma_start(
                    out=qh_sb[h][:, SBLK:S_LOC], in_=qh_d[h][:, SBLK:S_LOC]
                )

            # Prime slot 7 of every exp buffer with 1.0: the head-sum tree
            # then adds the softmax "+1" for free, and the slot is never
            # overwritten (exps write slots 0-6, mul writes 0:7).
            primed = []
            for _ in range(EXP_BUFS):
                t = exp_pool.tile([P, H, SBLK], f16, name="exp_sb")
                nc.gpsimd.memset(t[:, 7, :], 1.0)
                primed.append(t)

            AV_LAG = 3

            def emit_mul(tt_, exp_tile, rcp_tile):
                # normalize slots 0:7 IN PLACE with r broadcast over heads
                rcp_b = bass.AP(
                    tensor=rcp_tile.tensor,
                    offset=rcp_tile.offset,
                    ap=[rcp_tile.ap[0], [0, ND], rcp_tile.ap[1]],
                )
                nc.vector.tensor_mul(
                    exp_tile[:, 0:ND, :], exp_tile[:, 0:ND, :], rcp_b
                )

            def emit_av_pair(oacc4, tt_, exp_tile, rcp_tile, pr):
                for j in range(2):
                    h = 2 * pr + j
                    rhs = rcp_tile if h == 7 else exp_tile[:, h, :]
                    nc.tensor.matmul(
                        out=oacc4[:, pr, :][j * D : (j + 1) * D, :],
                        lhsT=v_sb[h][:, tt_, :],
                        rhs=rhs,
                        start=(tt_ == 0),
                        stop=(tt_ == NTT - 1),
                    )

            for sb in range(NSB):
                oacc4 = oacc_pool.tile([P, NPAIR, SBLK], f32, name="oacc")
                pend_mul = []  # (tt, exp_sb, rcp) awaiting normalize-mul
                pending = []   # (tt, exp_sb, rcp) mul done, awaiting AV
                for tt in range(NTT):
                    exp_sb = exp_pool.tile([P, H, SBLK], f16, name="exp_sb")
                    av = None
                    if len(pending) > AV_LAG - 2:
                        av = pending.pop(0)
                    # delta-head groups: (0,1), (2,3), (4,5), (6,)
                    for g in range(4):
                        heads = [2 * g, 2 * g + 1] if g < 3 else [6]
                        ps = score_pool.tile([P, 2, SBLK], f32, name="score")
                        for j, h in enumerate(heads):
                            nc.tensor.matmul(
                                out=ps[:, j, :],
                                lhsT=kh_sb[h][:, tt * P : (tt + 1) * P],
                                rhs=qh_sb[h][:, sb * SBLK : (sb + 1) * SBLK],
                                start=True,
                                stop=True,
                            )
                        if av is not None:
                            emit_av_pair(oacc4, av[0], av[1], av[2], g)
                        nc.scalar.activation(
                            out=exp_sb[:, heads[0] : heads[-1] + 1, :],
                            in_=ps[:, 0 : len(heads), :],
                            func=Exp,
                            scale=SCALE,
                        )
                    # head-sum tree: slot 7 == 1.0 supplies the softmax +1.
                    # z4 split in halves: z4a starts after exp group g1 (not
                    # g3), and the mul of the previous iteration fills the
                    # DVE until the last exp lands.
                    z4a = tmp_pool.tile([P, 2, SBLK], f16, name="z4a")
                    nc.vector.tensor_add(z4a, exp_sb[:, 0:2, :], exp_sb[:, 2:4, :])
                    if pend_mul:
                        t_, e_, r_ = pend_mul.pop(0)
                        emit_mul(t_, e_, r_)
                        pending.append((t_, e_, r_))
                    z4b = tmp_pool.tile([P, 2, SBLK], f16, name="z4b")
                    nc.vector.tensor_add(z4b, exp_sb[:, 4:6, :], exp_sb[:, 6:8, :])
                    z2 = tmp_pool.tile([P, 2, SBLK], f16, name="z2")
                    nc.vector.tensor_add(z2, z4a, z4b)
                    rcp = tmp_pool.tile([P, SBLK], f16, name="rcp")
                    nc.vector._custom_dve(
                        _RSUM, out=rcp, in0=z2[:, 0, :], in1=z2[:, 1, :],
                        s0=-0.23549792, s1=2.0017324,
                    )
                    pend_mul.append((tt, exp_sb, rcp))
                for t_, e_, r_ in pend_mul:
                    emit_mul(t_, e_, r_)
                    pending.append((t_, e_, r_))
                pend_mul = []
                for t_, e_, r_ in pending:
                    for pr in range(NPAIR):
                        emit_av_pair(oacc4, t_, e_, r_, pr)
                pending = []
                # drain per pair on ScalarE so output DMAs overlap the tail
                ot = outp.tile([P, NPAIR, SBLK], f32, name="ot")
                for pr in range(NPAIR):
                    nc.scalar.copy(out=ot[:, pr, :], in_=oacc4[:, pr, :])
                    nc.sync.dma_start(
                        out=out_d[pr, :, sb * SBLK : (sb + 1) * SBLK],
                        in_=ot[:, pr, :],
                    )

    nc.compile()
    return nc


def _get_nc():
    with _lock:
        if _cache["nc"] is None:
            _cache["nc"] = _build()
        return _cache["nc"]


def _prep_inputs(Q, K, V):
    Q = np.asarray(Q, dtype=np.float32)
    K = np.asarray(K, dtype=np.float32)
    V = np.asarray(V, dtype=np.float32)
    qt = np.ascontiguousarray(Q.transpose(0, 1, 3, 2)).astype(np.float16)  # [B,H,D,S]
    kb = K.astype(np.float16)  # [B,H,D,S]
    # stacked delta operands: KH[b,h] = [K_h; -K_7], QH[b,h] = [Q_h^T; Q_7^T]
    khat = np.empty((B, ND, P, S), dtype=np.float16)
    qhat = np.empty((B, ND, P, S), dtype=np.float16)
    for h in range(ND):
        khat[:, h, 0:D] = kb[:, h]
        khat[:, h, D:P] = -kb[:, 7]
        qhat[:, h, 0:D] = qt[:, h]
        qhat[:, h, D:P] = qt[:, 7]
    vp = np.ascontiguousarray(
        V.reshape(B, H, NTT, P, D).transpose(0, 1, 3, 2, 4)
    ).astype(np.float16)
    in_maps = []
    for c in range(N_CORES):
        b, sc = divmod(c, S_CHUNKS)
        in_maps.append(
            {
                "QH": np.ascontiguousarray(
                    qhat[b, :, :, sc * S_LOC : (sc + 1) * S_LOC]
                ),
                "KH": khat[b],
                "V": vp[b],
            }
        )
    return in_maps


def _assemble(results):
    # The reference output is a RAW reshape of contiguous [B, H, S, d] to
    # [B, S, H*d] (torch .view quirk), NOT a head-transpose.
    o_full = np.empty((B, H, S, D), dtype=np.float32)
    for c in range(N_CORES):
        b, sc = divmod(c, S_CHUNKS)
        shard = results[c]["OUT"]  # [NPAIR, 128, S_LOC]
        o_full[b, :, sc * S_LOC : (sc + 1) * S_LOC, :] = (
            shard.reshape(NPAIR, 2, D, S_LOC).transpose(0, 1, 3, 2).reshape(
                H, S_LOC, D
            )
        )
    return o_full.reshape(B, S, HIDDEN)


def run(Q, K, V, trace=False, **run_kwargs):
    nc = _get_nc()
    in_maps = _prep_inputs(Q, K, V)
    res = run_bass_kernel_spmd(
        nc, in_maps, core_ids=list(range(N_CORES)), trace=trace, **run_kwargs
    )
    return _assemble(res.results), res


def kernel(Q, K, V):
    prev = os.environ.get("BASS_NEVER_TRACE")
    os.environ["BASS_NEVER_TRACE"] = "1"
    try:
        out, _ = run(Q, K, V, trace=False)
    finally:
        if prev is None:
            os.environ.pop("BASS_NEVER_TRACE", None)
        else:
            os.environ["BASS_NEVER_TRACE"] = prev
    return out
